# revision 6
# baseline (speedup 1.0000x reference)
"""Trainium2 Bass kernel for segment_reduce (nn_BasicModel_37031208026271).

reference:
    value = poss_edge * weights[:, None]            # [E, 64]
    poss_node = segment_sum(value, edges[:, 0], N)  # [N, 64]
    poss_node = poss_node / neighbours_sum          # [N, 1] broadcast
    return poss_node, poss_edge

Strategy (node-parallel, no collectives needed):
  * Host: fold the division into per-edge scale w' = w / neighbours_sum[src],
    sort edges by src, shard NODES across the 8 cores (each core receives
    exactly the edges that point at its node range), bucket each core's edges
    into 128-node blocks, pad each block's edge list to a multiple of 128.
  * Device (identical SPMD program on 8 cores): for each 128-node block,
    accumulate over 128-edge chunks:
        M[k, p]  = (lo[k] == p) * w'[k]        (DVE tensor_scalar vs an iota row)
        psum    += value_chunk.T @ M            (PE matmul, value stationary)
    giving the transposed block [64 ch, 128 nodes] of the output, copied to
    SBUF and streamed back with one large DMA.
  * Host: transpose/unpermute the per-core outputs into [N, 64].

Per-core block slots use rank-matched chunk counts (each core maps its b-th
largest block to slot b; slot capacity = max over cores) so the single compiled
program wastes only a few % padding.
"""

import numpy as np

NUM_NODES = 100000
NUM_CH = 64  # NUM_CLASS + 1
N_CORES = 8
W = 128  # nodes per block (one-hot width / matmul free dim)
NB = 98  # blocks per core; 8*98*128 = 100352 >= 100000

_program_cache: dict = {}
_runner_cache: dict = {}
LAST_RUNNER = None  # _SpmdRunner of the most recent run (for benchmarking)


class _SpmdRunner:
    """Compile the Bass program once into a PJRT executable over the 8-core
    mesh; keep the jitted callable so repeated (timed) runs skip retracing."""

    def __init__(self, nc, n_cores):
        import jax
        import concourse.mybir as mybir
        from concourse import bass2jax
        from jax.experimental.shard_map import shard_map
        from jax.sharding import Mesh, PartitionSpec

        bass2jax.install_neuronx_cc_hook()
        self.nc = nc
        self.n_cores = n_cores
        partition_name = (nc.partition_id_tensor.name
                          if nc.partition_id_tensor else None)

        in_names: list[str] = []
        out_names: list[str] = []
        out_avals = []
        zero_outs: list[np.ndarray] = []
        for alloc in nc.m.functions[0].allocations:
            if not isinstance(alloc, mybir.MemoryLocationSet):
                continue
            name = alloc.memorylocations[0].name
            if alloc.kind == "ExternalInput":
                if name != partition_name:
                    in_names.append(name)
            elif alloc.kind == "ExternalOutput":
                shape = tuple(alloc.tensor_shape)
                dtype = mybir.dt.np(alloc.dtype)
                out_names.append(name)
                out_avals.append(jax.core.ShapedArray(shape, dtype))
                zero_outs.append(np.zeros(shape, dtype))
        self.n_params = len(in_names)
        self.param_names = list(in_names)
        self.out_names = out_names
        self.out_avals = out_avals
        self.zero_outs = zero_outs
        in_names.extend(out_names)
        if partition_name is not None:
            in_names.append(partition_name)

        def _body(*args):
            operands = list(args)
            if partition_name is not None:
                operands.append(bass2jax.partition_id_tensor())
            outs = bass2jax._bass_exec_p.bind(
                *operands,
                out_avals=tuple(out_avals),
                in_names=tuple(in_names),
                out_names=tuple(out_names),
                lowering_input_output_aliases=(),
                sim_require_finite=True,
                sim_require_nnan=True,
                nc=nc,
            )
            return tuple(outs)

        devices = jax.devices()[:n_cores]
        assert len(devices) == n_cores
        self.mesh = Mesh(np.asarray(devices), ("core",))
        n_in = self.n_params + len(out_names)
        self.fn = jax.jit(
            shard_map(
                _body,
                mesh=self.mesh,
                in_specs=(PartitionSpec("core"),) * n_in,
                out_specs=(PartitionSpec("core"),) * len(out_names),
                check_rep=False,
            ),
            keep_unused=True,
        )
        self._dev_args = None

    def _concat_args(self, in_maps):
        concat_in = [
            np.concatenate([np.asarray(m[name]) for m in in_maps], axis=0)
            for name in self.param_names
        ]
        concat_zeros = [
            np.zeros((self.n_cores * z.shape[0], *z.shape[1:]), z.dtype)
            for z in self.zero_outs
        ]
        return concat_in + concat_zeros

    def run(self, in_maps):
        args = self._concat_args(in_maps)
        self._last_args = args
        out_arrs = self.fn(*args)
        return [
            {
                name: np.asarray(out_arrs[i]).reshape(
                    self.n_cores, *self.out_avals[i].shape)[c]
                for i, name in enumerate(self.out_names)
            }
            for c in range(self.n_cores)
        ]

    def bench(self, iters=20, warmup=3):
        """Time repeated executions with device-resident inputs."""
        import time
        import jax
        from jax.sharding import NamedSharding, PartitionSpec

        if self._dev_args is None:
            sharding = NamedSharding(self.mesh, PartitionSpec("core"))
            self._dev_args = [jax.device_put(a, sharding)
                              for a in self._last_args]
        for _ in range(warmup):
            jax.block_until_ready(self.fn(*self._dev_args))
        times = []
        for _ in range(iters):
            t0 = time.perf_counter()
            jax.block_until_ready(self.fn(*self._dev_args))
            times.append(time.perf_counter() - t0)
        return times


def _build_program(c_list, w, num_ch):
    """Compile the SPMD Bass program for per-slot chunk counts c_list."""
    import concourse.bacc as bacc
    import concourse.tile as tile
    import concourse.mybir as mybir

    key = (tuple(c_list), w, num_ch)
    if key in _program_cache:
        return _program_cache[key]

    nb = len(c_list)
    tot = int(sum(c_list))
    f32 = mybir.dt.float32

    nc = bacc.Bacc("TRN2", target_bir_lowering=False, debug=False,
                   num_devices=N_CORES)
    val = nc.dram_tensor("val", [128, tot * num_ch], f32,
                         kind="ExternalInput").ap()
    meta = nc.dram_tensor("meta", [128, tot * 2], f32,
                          kind="ExternalInput").ap()
    iota = nc.dram_tensor("iota", [128, w], f32, kind="ExternalInput").ap()
    out = nc.dram_tensor("out", [num_ch, nb * w], f32,
                         kind="ExternalOutput").ap()

    with tile.TileContext(nc) as tc:
        with (
            tc.tile_pool(name="const", bufs=1) as constp,
            tc.tile_pool(name="metap", bufs=1) as metap,
            tc.tile_pool(name="valp", bufs=3) as valp,
            tc.tile_pool(name="mp", bufs=6) as mp,
            tc.tile_pool(name="outp", bufs=1) as outp,
            tc.tile_pool(name="ps", bufs=6, space="PSUM") as psp,
        ):
            iota_t = constp.tile([128, w], f32)
            nc.sync.dma_start(iota_t[:], iota[:])
            meta_t = metap.tile([128, tot * 2], f32)
            nc.sync.dma_start(meta_t[:], meta[:])
            out_sb = outp.tile([num_ch, nb * w], f32)

            off = 0
            for b in range(nb):
                cb = int(c_list[b])
                vt = valp.tile([128, cb * num_ch], f32, tag="valtile")
                nc.sync.dma_start(
                    vt[:], val[:, off * num_ch:(off + cb) * num_ch])
                ps = psp.tile([num_ch, w], f32, tag="pstile")
                for c in range(cb):
                    j = off + c
                    m = mp.tile([128, w], f32, tag="onehot")
                    nc.vector.tensor_scalar(
                        out=m[:],
                        in0=iota_t[:],
                        scalar1=meta_t[:, 2 * j:2 * j + 1],
                        scalar2=meta_t[:, 2 * j + 1:2 * j + 2],
                        op0=mybir.AluOpType.is_equal,
                        op1=mybir.AluOpType.mult,
                    )
                    nc.tensor.matmul(
                        out=ps[:],
                        lhsT=vt[:, c * num_ch:(c + 1) * num_ch],
                        rhs=m[:],
                        start=(c == 0),
                        stop=(c == cb - 1),
                    )
                nc.scalar.copy(out_sb[:, b * w:(b + 1) * w], ps[:])
                off += cb

            nc.sync.dma_start(out[:], out_sb[:])

    nc.compile()
    _program_cache[key] = nc
    return nc


def _prepare_core(src_local, sperm, poss_edge, wprime, nb, w, num_ch):
    """Per-core host prep. src_local: sorted local node ids of this core's
    edges; sperm: global edge ids in the same order. Returns (counts, order,
    per-rank chunk counts, and a closure-ready dict of the raw pieces)."""
    blk = src_local // w                      # block id per edge, sorted
    counts = np.bincount(blk, minlength=nb)   # edges per block
    order = np.argsort(-counts, kind="stable").astype(np.int64)  # slot -> blk
    chunks = (counts[order] + 127) // 128     # per-slot needed chunks
    return blk, counts, order, chunks


def _kernel_impl(edges, weights, poss_edge, neighbours_sum,
                 num_nodes, n_cores, nb, w, num_ch):
    edges = np.asarray(edges)
    weights = np.asarray(weights, dtype=np.float32)
    poss_edge = np.asarray(poss_edge, dtype=np.float32)
    neighbours_sum = np.asarray(neighbours_sum, dtype=np.float32)

    nodes_per_core = nb * w
    src = np.ascontiguousarray(edges[:, 0]).astype(np.int64)
    wprime = weights / neighbours_sum[:, 0][src]

    perm = np.argsort(src, kind="stable")
    ssrc = src[perm]
    core_bounds = np.searchsorted(
        ssrc, np.arange(0, n_cores + 1) * nodes_per_core)

    per_core = []
    all_chunks = []
    for r in range(n_cores):
        lo_i, hi_i = int(core_bounds[r]), int(core_bounds[r + 1])
        sperm = perm[lo_i:hi_i]
        src_local = ssrc[lo_i:hi_i] - r * nodes_per_core
        blk, counts, order, chunks = _prepare_core(
            src_local, sperm, poss_edge, wprime, nb, w, num_ch)
        per_core.append((sperm, src_local, blk, counts, order))
        all_chunks.append(chunks)

    c_list = np.maximum(np.max(np.stack(all_chunks), axis=0), 1)
    tot = int(c_list.sum())
    slot_off = np.concatenate([[0], np.cumsum(c_list)]).astype(np.int64)

    nc = _build_program(tuple(int(x) for x in c_list), w, num_ch)

    iota_np = np.ascontiguousarray(
        np.broadcast_to(np.arange(w, dtype=np.float32), (128, w)))

    in_maps = []
    for r in range(n_cores):
        sperm, src_local, blk, counts, order = per_core[r]
        inv_order = np.empty(nb, dtype=np.int64)
        inv_order[order] = np.arange(nb)
        block_start = np.concatenate([[0], np.cumsum(counts)]).astype(np.int64)
        within = np.arange(len(src_local), dtype=np.int64) - block_start[blk]
        dest_chunk = slot_off[inv_order[blk]] + within // 128
        dest_lane = within % 128

        val_pad = np.zeros((tot, 128, num_ch), dtype=np.float32)
        val_pad[dest_chunk, dest_lane] = poss_edge[sperm]
        meta_pad = np.zeros((tot, 128, 2), dtype=np.float32)
        meta_pad[dest_chunk, dest_lane, 0] = (src_local % w).astype(np.float32)
        meta_pad[dest_chunk, dest_lane, 1] = wprime[sperm]

        in_maps.append({
            "val": np.ascontiguousarray(
                val_pad.transpose(1, 0, 2)).reshape(128, tot * num_ch),
            "meta": np.ascontiguousarray(
                meta_pad.transpose(1, 0, 2)).reshape(128, tot * 2),
            "iota": iota_np,
        })

    global LAST_RUNNER
    key = id(nc)
    if key not in _runner_cache:
        _runner_cache[key] = _SpmdRunner(nc, n_cores)
    runner = _runner_cache[key]
    LAST_RUNNER = runner
    results = runner.run(in_maps)

    pieces = []
    for r in range(n_cores):
        _, _, _, _, order = per_core[r]
        o = np.asarray(results[r]["out"])          # [num_ch, nb*w]
        o = o.reshape(num_ch, nb, w).transpose(1, 2, 0)  # [slot, lane, ch]
        blocks = np.empty_like(o)
        blocks[order] = o                               # block b at blocks[b]
        pieces.append(blocks.reshape(nodes_per_core, num_ch))

    poss_node = np.concatenate(pieces, axis=0)[:num_nodes]
    return poss_node, poss_edge


def kernel(edges, weights, poss_edge, neighbours_sum):
    return _kernel_impl(edges, weights, poss_edge, neighbours_sum,
                        NUM_NODES, N_CORES, NB, W, NUM_CH)
